# revision 69
# baseline (speedup 1.0000x reference)
"""Sliding-window attention (WINDOW=129) Trainium2 Bass kernel.

Problem: x[B=2, N=2048, C=768] -> qkv proj -> 12-head sliding-window
attention (half-window 64) -> output proj + bias.

Sharding: sequence-parallel over 8 cores: core c handles batch b = c//4,
query chunk s = c%4 (512 queries), with a 64-row halo each side for K/V.
Weights replicated. Each core computes its 512 output rows completely;
host concatenates. No collectives.

Per-core pipeline (matmul operands fp16, psum f32):
  Loads: wqk chunk 0 first, then x, the rest of wqk, wv, wp, spread over
    the SP/ACT sequencers in priority order (HWDGE issue and the DMA
    engines serialize, so few wide transfers in consumption order);
    amask/valid/bias on gpsimd SWDGE. Zeros x zeros dummy matmuls hold
    the PE p-state ramp through the load window.
  qkT [e, n]: two 64-row head-groups per M=128 matmul; the lower group
    is one psum->sbuf fp16 convert copy (ACT), the upper group converts
    in place (DVE) then takes a half-cost fp16 stream_shuffle to
    partition base 0 (matmul OPERANDS at partition base 64 fault this
    device - HW-verified - and stream_shuffle cannot convert dtypes).
  v -> vaug [n, 12*(64+1)]; the per-head column 64 holds WS x the key
    validity flag, which both excludes out-of-sequence keys from the
    softmax denominator (their v is already 0 from x zero-padding) and
    cancels the WS prescale of wv.
  scores sT[k, q] per key-tile via K=64 fp16 matmuls; the band mask is
    accumulated INSIDE the psum group as an fp8e5m2 DoubleRow identity
    matmul adding amask (0 in band, -8192 outside, exact in e5m2);
    exp(EXPSCALE * psum) on ACT underflows masked entries to 0, and
    EXPSCALE folds out the WS prescale of wq/wk. The first two key
    tiles' score tiles are interleaved into the v loop so their exps
    run on ACT during the v phase.
  AV with q on the output partition dim: out[q, 65] = pT_slice.T @ vaug;
    column 64 gives WS x the denominator; DVE reciprocal + broadcast
    multiply normalizes; PE transpose -> attnT [c, n]. Rounds are
    software-pipelined one round deep (scores r+2 before av r) so av
    never waits on ACT exps.
  proj matmul + DVE bias add; both column-chunk stores issue on the
    idle SP sequencer (ACT's queue would delay them).
"""

import numpy as np
import ml_dtypes

import concourse.bass as bass
import concourse.tile as tile
from concourse import bacc, mybir
from concourse._compat import with_exitstack
from concourse.masks import make_identity

B, N, C = 2, 2048, 768
H, D = 12, 64
HALF = 64            # half window
NCORES = 8
CHUNK = 512          # queries per core
NK = CHUNK + 2 * HALF  # 640 rows incl halo
SCALE = D ** -0.5
WS = 4.0             # host prescale on wq/wk/wv for fp8 range
EXPSCALE = SCALE / (WS * WS)   # folds the wq*wk prescale back out
MASKVAL = -8192.0    # additive out-of-band mask, exact in fp8e5m2
FP8_QKV = False      # fp8e4m3 DoubleRow for the qkT / v projections
FP8_MASK = True      # fp8e5m2 DoubleRow identity matmul for the band mask

F16 = mybir.dt.float16
F32 = mybir.dt.float32
F8 = mybir.dt.float8e4
F8E5 = mybir.dt.float8e5
DR = mybir.MatmulPerfMode.DoubleRow
NP_F8 = ml_dtypes.float8_e4m3
NP_F8E5 = ml_dtypes.float8_e5m2
IDENT32 = list(range(32))


@with_exitstack
def attn_core_kernel(ctx, tc, outs, ins, repeat=1):
    nc = tc.nc
    out_ap = outs["out"]
    xT, wqkT, wvT, wpT, biasR, amask, valid = (
        ins["xT"], ins["wqkT"], ins["wvT"], ins["wpT"], ins["biasR"],
        ins["amask"], ins["valid"],
    )

    consts = ctx.enter_context(tc.tile_pool(name="consts", bufs=1))
    ppool = ctx.enter_context(tc.tile_pool(name="ps", bufs=2, space="PSUM"))
    scpool = ctx.enter_context(tc.tile_pool(name="scp", bufs=2, space="PSUM"))
    ptpool = ctx.enter_context(tc.tile_pool(name="pt", bufs=18))
    rcpool = ctx.enter_context(tc.tile_pool(name="rc", bufs=4))
    aqpool = ctx.enter_context(tc.tile_pool(name="aq", bufs=2))
    outpool = ctx.enter_context(tc.tile_pool(name="ob", bufs=3))
    shpool = ctx.enter_context(tc.tile_pool(name="sh", bufs=4))

    QKV_DT = F8 if FP8_QKV else F16
    xT_sb = consts.tile([128, 6, NK], QKV_DT)
    wqk_sb = consts.tile([128, 6, 1536], QKV_DT)
    wv_sb = consts.tile([128, 6, 768], QKV_DT)
    wp_sb = consts.tile([128, 6, 768], F16)
    amask_sb = consts.tile([128, 2, 512], F8E5 if FP8_MASK else F16)
    valid_sb = consts.tile([128, 5], F16)
    bias_sb = consts.tile([128, 768], F32)
    qk_sb = consts.tile([64, 24, NK], F16)      # [d, group, n]; q: h, k: 12+h
    vaug_sb = consts.tile([128, 5, H * 65], F16)  # [n-tile, head*(64+4*valid)]
    attnT_sb = consts.tile([128, 6, CHUNK], F16)  # [c-tile, n]
    ident_sb = consts.tile([128, 128], F16)
    ident8_sb = consts.tile([128, 2, 128], F8E5)  # DoubleRow identity, slot1=0
    junk_sb = consts.tile([128, 512], F8E5)
    ones_set = [False]

    xT3 = xT.rearrange("(t p) n -> p t n", p=128)
    wqk3 = wqkT.rearrange("(t p) e -> p t e", p=128)
    wv3 = wvT.rearrange("(t p) e -> p t e", p=128)
    wp3 = wpT.rearrange("(t p) e -> p t e", p=128)

    def loads():
        if not ones_set[0]:
            # junk memset first (on Pool, the fastest-starting engine):
            # it alone gates the warmup matmuls
            nc.gpsimd.memset(junk_sb[:], 0.0)
            # dummy matmuls (zeros x zeros): keep PE busy through the
            # p-state ramp while the input DMAs stream in
            for _wi in range(10):
                psw = ppool.tile([128, 512], F32, tag="mm", bufs=3)
                nc.tensor.matmul(psw[:], junk_sb[:, 0:128], junk_sb[:],
                                 start=True, stop=True)
            nc.gpsimd.memset(ident8_sb[:, 1, :], 0.0)
            make_identity(nc, ident8_sb[:, 0, :])
            make_identity(nc, ident_sb[:])
            ones_set[0] = True
        # priority order on the serial HWDGE/DMA-engine path: wqk chunk 0
        # (with the first x half right behind) unblocks the first qkT
        # pairs earliest; then the rest of wqk, wv, wp
        nc.sync.dma_start(wqk_sb[:, :, 0:256], wqk3[:, :, 0:256])
        nc.scalar.dma_start(xT_sb[:, 0:3, :], xT3[:, 0:3, :])
        nc.sync.dma_start(xT_sb[:, 3:6, :], xT3[:, 3:6, :])
        nc.scalar.dma_start(wqk_sb[:, :, 256:512], wqk3[:, :, 256:512])
        nc.scalar.dma_start(wqk_sb[:, :, 512:1024], wqk3[:, :, 512:1024])
        nc.sync.dma_start(wqk_sb[:, :, 1024:1536], wqk3[:, :, 1024:1536])
        nc.scalar.dma_start(wv_sb[:], wv3[:])
        nc.sync.dma_start(wp_sb[:, 0:3, :], wp3[:, 0:3, :])
        nc.scalar.dma_start(wp_sb[:, 3:6, :], wp3[:, 3:6, :])
        nc.gpsimd.dma_start(valid_sb[:], valid)
        nc.gpsimd.dma_start(amask_sb[:], amask)
        nc.gpsimd.dma_start(bias_sb[:], biasR[0:1, :].to_broadcast((128, 768)))
        # per-head validity column of vaug (offset 64, stride 65); on
        # gpsimd so the late-landing valid DMA can't head-of-line block
        # the DVE program
        va = vaug_sb.rearrange("p t (h u) -> p t h u", u=65)
        for kt in range(5):
            nc.gpsimd.tensor_copy(
                out=va[:, kt, :, 64],
                in_=valid_sb[:, kt:kt + 1].to_broadcast((128, H)),
            )

    def qkv():
        # qkT: two 64-row head-groups per M=128 DoubleRow matmul
        for pe in range(12):       # pair index: groups 2pe, 2pe+1
            # q (pe<6) only ever read for q_loc in [64, 576)
            chunks = ((64, 512),) if pe < 6 else ((0, 512), (512, 128))
            for c0, w in chunks:
                d0 = c0 - 64 if pe < 6 else c0
                ps = ppool.tile([128, 512], F32, tag="mm", bufs=3)
                if FP8_QKV:
                    for ct in range(3):
                        nc.tensor.matmul(
                            ps[:, :w],
                            wqk_sb[:, 2 * ct:2 * ct + 2, pe * 128:(pe + 1) * 128],
                            xT_sb[:, 2 * ct:2 * ct + 2, c0:c0 + w],
                            start=(ct == 0), stop=(ct == 2), perf_mode=DR,
                        )
                else:
                    for ct in range(6):
                        nc.tensor.matmul(
                            ps[:, :w],
                            wqk_sb[:, ct, pe * 128:(pe + 1) * 128],
                            xT_sb[:, ct, c0:c0 + w],
                            start=(ct == 0), stop=(ct == 5),
                        )
                # lower group: one psum->sbuf fp16 convert copy. Upper
                # group: fp16 convert copy in place (same partitions),
                # then a half-cost fp16 stream_shuffle to partition base 0
                # (matmul operands at partition base 64 fault this device,
                # and stream_shuffle cannot convert dtypes).
                nc.scalar.copy(out=qk_sb[:, 2 * pe, d0:d0 + w],
                               in_=ps[0:64, :w])
                sh = shpool.tile([128, 512], F16, tag="sh")
                nc.vector.tensor_copy(out=sh[64:128, :w], in_=ps[64:128, :w])
                nc.vector.stream_shuffle(qk_sb[:, 2 * pe + 1, d0:d0 + w],
                                         sh[64:128, :w], IDENT32)
        # score tiles for the first two key tiles are interleaved with
        # the v chunks so their exps occupy ACT during the v phase (and
        # PE never runs more than the psum pipeline ahead of ACT)
        early_sc = [(kt, hp) for hp in range(6) for kt in (0, 1)]
        # v -> vaug (strided per-head destination); copies split DVE/ACT
        va = vaug_sb.rearrange("p t (h u) -> p t h u", u=65)
        for nt in range(5):
            for c0, w, h0, nh in ((0, 512, 0, 8), (512, 256, 8, 4)):
                if early_sc:
                    scores_tile(*early_sc.pop(0))
                if early_sc and len(early_sc) > 8:
                    scores_tile(*early_sc.pop(0))
                ps = ppool.tile([128, 512], F32, tag="mm", bufs=3)
                if FP8_QKV:
                    for ct in range(3):
                        nc.tensor.matmul(
                            ps[:, :w],
                            xT_sb[:, 2 * ct:2 * ct + 2, nt * 128:(nt + 1) * 128],
                            wv_sb[:, 2 * ct:2 * ct + 2, c0:c0 + w],
                            start=(ct == 0), stop=(ct == 2), perf_mode=DR,
                        )
                else:
                    for ct in range(6):
                        nc.tensor.matmul(
                            ps[:, :w],
                            xT_sb[:, ct, nt * 128:(nt + 1) * 128],
                            wv_sb[:, ct, c0:c0 + w],
                            start=(ct == 0), stop=(ct == 5),
                        )
                src = ps[:, :w].rearrange("p (h d) -> p h d", d=64)
                dst = va[:, nt, h0:h0 + nh, 0:64]
                if c0 == 0:
                    nc.vector.tensor_copy(out=dst, in_=src)
                else:
                    nc.scalar.copy(out=dst, in_=src)
        for kt_hp in early_sc:
            scores_tile(*kt_hp)
        early_sc.clear()

    pt_tiles = {}

    def scores_tile(kt, hp):
        # band mask accumulated into psum via an fp8e5m2 DoubleRow
        # identity matmul (amask: 0 in band, -8192 outside -> exp gives 0)
        cq0, cq1 = (128, 256) if kt == 0 else ((0, 128) if kt == 4 else (0, 256))
        if True:
            sc = scpool.tile([128, 512], F32, tag="sc")
            for j2 in range(2):
                h = 2 * hp + j2
                lhsT = qk_sb[:, 12 + h, kt * 128:kt * 128 + 128]
                rhs = qk_sb[:, h, kt * 128 - 128 + cq0:kt * 128 - 128 + cq1]
                csl = slice(256 * j2 + cq0, 256 * j2 + cq1)
                nc.tensor.matmul(sc[:, csl], lhsT, rhs, start=True, stop=False)
                if FP8_MASK:
                    nc.tensor.matmul(sc[:, csl], ident8_sb[:],
                                     amask_sb[:, :, csl],
                                     start=False, stop=True, perf_mode=DR)
                else:
                    nc.tensor.matmul(sc[:, csl], ident_sb[:],
                                     amask_sb[:, 0, csl],
                                     start=False, stop=True)
            pt = ptpool.tile([128, 512], F16, tag="pt")
            sc2 = sc.rearrange("p (h q) -> p h q", h=2)
            pt2 = pt.rearrange("p (h q) -> p h q", h=2)
            nc.scalar.activation(out=pt2[:, :, cq0:cq1], in_=sc2[:, :, cq0:cq1],
                                 func=mybir.ActivationFunctionType.Exp,
                                 scale=EXPSCALE)
            pt_tiles[(kt, hp)] = pt

    def scores_kt(kt):
        for hp in range(6):
            scores_tile(kt, hp)

    aq_tiles = {}

    def av_core(r):
        va = vaug_sb.rearrange("p t (h u) -> p t h u", u=65)
        aq = aqpool.tile([128, 768], F16, tag="aq")
        aq_tiles[r] = aq
        aq3 = aq.rearrange("p (h d) -> p h d", d=64)
        for hg in range(3):
            av = ppool.tile([128, 260], F32, tag="av", bufs=3)
            av3 = av.rearrange("p (h u) -> p h u", u=65)
            for j in range(4):
                h = 4 * hg + j
                for ki, kt in ((0, r), (1, r + 1)):
                    col0 = 128 if ki == 0 else 0
                    pt = pt_tiles[(kt, h // 2)]
                    lhsT = pt[:, 256 * (h % 2) + col0:256 * (h % 2) + col0 + 128]
                    nc.tensor.matmul(av3[:, j, :], lhsT, va[:, kt, h, :],
                                     start=(ki == 0), stop=(ki == 1))
            rc = rcpool.tile([128, 4], F32, tag="rc")
            nc.vector.reciprocal(rc[:], av3[:, :, 64])
            nc.vector.tensor_tensor(
                aq3[:, 4 * hg:4 * hg + 4, :],
                av3[:, :, 0:64],
                rc[:, :, None].to_broadcast((128, 4, 64)),
                mybir.AluOpType.mult,
            )
    def av_tr(r):
        # transpose [q, c] -> attnT [c, q] per head pair; batched copies
        aq = aq_tiles.pop(r)
        qsl = slice(128 * r, 128 * r + 128)
        tr = ppool.tile([128, 6, 128], F16, tag="mm", bufs=3)
        for hp in range(6):
            nc.tensor.transpose(tr[:, hp, :], aq[:, 128 * hp:128 * hp + 128],
                                ident_sb[:])
        nc.vector.tensor_copy(out=attnT_sb[:, 0:3, qsl], in_=tr[:, 0:3, :])
        nc.scalar.copy(out=attnT_sb[:, 3:6, qsl], in_=tr[:, 3:6, :])

    def av_r(r):
        av_core(r)
        av_tr(r)

    def proj_r(r):
        rsl = slice(128 * r, 128 * r + 128)
        ob = outpool.tile([128, 768], F32, tag="ob")
        chunks = ((0, 512), (512, 256))
        for c0, w in chunks:
            ps = ppool.tile([128, 512], F32, tag="mm", bufs=3)
            for ct in range(6):
                nc.tensor.matmul(
                    ps[:, :w],
                    attnT_sb[:, ct, 128 * r:128 * r + 128],
                    wp_sb[:, ct, c0:c0 + w],
                    start=(ct == 0), stop=(ct == 5),
                )
            nc.vector.tensor_add(out=ob[:, c0:c0 + w], in0=ps[:, :w],
                                 in1=bias_sb[:, c0:c0 + w])
            eng = nc.scalar if c0 == 256 else nc.sync
            eng.dma_start(out_ap[rsl, c0:c0 + w], ob[:, c0:c0 + w])

    for _rep in range(repeat):
        pt_tiles.clear()
        loads()
        qkv()
        # software-pipelined one round deep: scores of round r+2 are
        # emitted before av of round r, so every av reads exps that were
        # queued a full round earlier and PE never waits on ACT
        scores_kt(2)
        av_r(0)
        scores_kt(3)
        proj_r(0)
        av_r(1)
        scores_kt(4)
        proj_r(1)
        av_core(2)
        av_core(3)
        av_tr(2)
        av_tr(3)
        proj_r(2)
        proj_r(3)


def build_nc(repeat=1):
    nc = bacc.Bacc("TRN2", target_bir_lowering=False, debug=False)
    ins = {
        "xT": nc.dram_tensor("xT", [C, NK], F8 if FP8_QKV else F16, kind="ExternalInput").ap(),
        "wqkT": nc.dram_tensor("wqkT", [C, 2 * C], F8 if FP8_QKV else F16, kind="ExternalInput").ap(),
        "wvT": nc.dram_tensor("wvT", [C, C], F8 if FP8_QKV else F16, kind="ExternalInput").ap(),
        "wpT": nc.dram_tensor("wpT", [C, C], F16, kind="ExternalInput").ap(),
        "biasR": nc.dram_tensor("biasR", [1, C], F32, kind="ExternalInput").ap(),
        "amask": nc.dram_tensor("amask", [128, 2, 512], F8E5 if FP8_MASK else F16,
                                kind="ExternalInput").ap(),
        "valid": nc.dram_tensor("valid", [128, 5], F16, kind="ExternalInput").ap(),
    }
    outs = {"out": nc.dram_tensor("out", [CHUNK, C], F32, kind="ExternalOutput").ap()}
    with tile.TileContext(nc) as tc:
        attn_core_kernel(tc, outs, ins, repeat=repeat)
    nc.finalize()
    return nc


def make_core_inputs(x, w_qkv, w_proj, b_proj):
    """Build the 8 per-core input maps from full inputs."""
    x = np.asarray(x, dtype=np.float32)
    w_qkv = np.asarray(w_qkv, dtype=np.float32)
    w_proj = np.asarray(w_proj, dtype=np.float32)
    b_proj = np.asarray(b_proj, dtype=np.float32)

    # wq/wk/wv prescaled by WS=4 so fp8e4m3 sees values in its normal range
    wqk = np.concatenate([w_qkv[:C], w_qkv[C:2 * C]], axis=0) * WS
    np_qkv = NP_F8 if FP8_QKV else np.float16
    wqkT = np.ascontiguousarray(wqk.T).astype(np_qkv)
    wvT = np.ascontiguousarray(w_qkv[2 * C:].T * WS).astype(np_qkv)
    wpT = np.ascontiguousarray(w_proj.T).astype(np.float16)
    biasR = b_proj.reshape(1, C).astype(np.float32)

    # additive band mask (two identical 256-query halves, both DoubleRow
    # slots): 0 iff 0 <= cq - k <= 128 else MASKVAL
    k = np.arange(128)[:, None]
    cq = np.arange(256)[None, :]
    band = (cq - k >= 0) & (cq - k <= 128)
    half_tile = np.where(band, 0.0, MASKVAL).astype(np.float32)
    amask = np.ascontiguousarray(np.broadcast_to(
        np.concatenate([half_tile, half_tile], axis=1)[:, None, :],
        (128, 2, 512),
    )).astype(NP_F8E5 if FP8_MASK else np.float16)

    in_maps = []
    for c in range(NCORES):
        b, s = divmod(c, 4)
        lo = s * CHUNK - HALF
        hi = s * CHUNK + CHUNK + HALF
        xs = np.zeros((NK, C), dtype=np.float32)
        s0, s1 = max(lo, 0), min(hi, N)
        xs[s0 - lo:s1 - lo] = x[b, s0:s1]
        xT = np.ascontiguousarray(xs.T).astype(np_qkv)

        # per-key validity flag, [128 partitions, 5 key tiles]; value WS
        # cancels the wv prescale through the softmax denominator
        p = np.arange(128)[:, None]
        kt = np.arange(5)[None, :]
        key_seq = s * CHUNK - HALF + 128 * kt + p
        valid = (((key_seq >= 0) & (key_seq < N)) * WS).astype(np.float16)

        in_maps.append({
            "xT": xT, "wqkT": wqkT, "wvT": wvT, "wpT": wpT,
            "biasR": biasR, "amask": amask, "valid": valid,
        })
    return in_maps


_NC_CACHE = None


def kernel(x, w_qkv, w_proj, b_proj):
    from concourse.bass_utils import run_bass_kernel_spmd

    global _NC_CACHE
    if _NC_CACHE is None:
        _NC_CACHE = build_nc()
    in_maps = make_core_inputs(x, w_qkv, w_proj, b_proj)
    res = run_bass_kernel_spmd(_NC_CACHE, in_maps, core_ids=list(range(NCORES)))
    out = np.empty((B, N, C), dtype=np.float32)
    for c in range(NCORES):
        b, s = divmod(c, 4)
        out[b, s * CHUNK:(s + 1) * CHUNK] = res.results[c]["out"]
    return out
